# revision 21
# baseline (speedup 1.0000x reference)
"""Multi-head self-attention (B=4, S=2048, E=1024, H=16, D=64) on 8 TRN2 cores.

Sharding: core c handles batch b = c//2 and head-group hg = c%2 (8 of 16 heads).
QKV weights column-parallel, proj row-parallel (Megatron); the two cores
sharing a batch produce partial proj outputs that are summed on the host.

Device layout (per core):
  xt (x[b].T, [E=1024, S=2048] fp16) and the concatenated qkv weight
  [E, 1536] are loaded ONCE into SBUF with 4KB/3KB DMA lines and stay
  resident (the v1 kernel re-fetched xt per projection unit: 42MB -> 18MB).
  qT/kT = (x W)^T per head-pair  [128 feats, pair, 2048]
  scores computed transposed:     S_T[k, q] = kT.T @ qT (pair-packed on
                                  the 64-wide contraction via base_partition)
  softmax without max-subtraction (scores ~N(0,1); exp(s/8-4) stays in
  fp16 range), denominator fused into the AV matmul via a ones-augmented
  [V | 1] stationary; O^T accumulated per head in [65, 512] PSUM tiles.
  norm: fp16 den row (keeps the PE in fp16 mode - fp32r mode switches
  cost ~0.6us PE drain each), PE ones-broadcast, DVE reciprocal + muls.
  proj: out[s, n] = attnT.T @ Wp, accumulated over feature pairs.

Scheduling: one global stream of attention (block, kc) chunks with lag-2
AV (scores run two chunks ahead of AV so the scalar-engine exp latency
and semaphore round-trips are fully hidden), 3-deep PSUM rotation for
the scores->exp pipe, and atomic 8-matmul cover units (QKV / output
projections) placed so every block has PE work while exp streams;
deferred q-column units keep the final head-pair covered too.
"""
import numpy as np

B, S, E = 4, 2048, 1024
H, D = 16, 64
HLOC = 8          # heads per core
FEAT = HLOC * D   # 512 per-core q/k/v features
NCORES = 8

_CACHE = {}


def _build_program(has_bqkv, has_bp):
    import concourse.bass as bass
    import concourse.mybir as mybir
    from concourse import bacc
    from concourse.tile import TileContext

    F32R = mybir.dt.float32r
    F32 = mybir.dt.float32
    F16 = mybir.dt.float16
    AF = mybir.ActivationFunctionType

    nc = bacc.Bacc("TRN2", target_bir_lowering=False, num_devices=NCORES)

    xt = nc.dram_tensor("xt", [E, S], F16, kind="ExternalInput")
    wqkv = nc.dram_tensor("wqkv", [E, 3 * FEAT], F16, kind="ExternalInput")
    wp = nc.dram_tensor("wp", [FEAT, E], F16, kind="ExternalInput")
    out = nc.dram_tensor("out", [S, E], F32, kind="ExternalOutput")
    if has_bqkv:
        bq_d = nc.dram_tensor("bq", [128, 4], F32, kind="ExternalInput")
        bk_d = nc.dram_tensor("bk", [128, 4], F32, kind="ExternalInput")
        bv_d = nc.dram_tensor("bv", [1, FEAT], F16, kind="ExternalInput")
    if has_bp:
        bp_d = nc.dram_tensor("bp", [1, E], F16, kind="ExternalInput")

    EC = E // 128      # 8 e-chunks (contraction for qkv)
    SC = S // 512      # 4 s-chunks of 512 (q blocks)
    KC = S // 128      # 16 k-chunks of 128
    NP = HLOC // 2     # 4 head pairs
    CH = 512
    NCH = S // CH

    xt_v = xt.ap().rearrange("(c p) s -> p c s", p=128)
    wqkv_v = wqkv.ap().rearrange("(c p) f -> p c f", p=128)
    wp_v = wp.ap().rearrange("(c p) n -> p c n", p=128)

    with TileContext(nc) as tc:
      from contextlib import ExitStack
      from collections import deque
      with ExitStack() as es:
        pp = es.enter_context(tc.tile_pool(name="persist", bufs=1))
        pqk = es.enter_context(tc.tile_pool(name="pqk", bufs=1))
        pB = es.enter_context(tc.tile_pool(name="pB", bufs=1))
        pout = es.enter_context(tc.tile_pool(name="pout", bufs=3))
        psS = es.enter_context(tc.tile_pool(name="psS", bufs=3, space="PSUM"))
        psO = es.enter_context(tc.tile_pool(name="psO", bufs=1, space="PSUM"))

        xt_sb = pp.tile([128, EC, S], F16)
        w_sb = pp.tile([128, EC, 3 * FEAT], F16)   # [q | k | v] features
        wp_sb = pp.tile([128, NP, E], F16)
        qT = pqk.tile([128, NP, S], F16)     # [feat128, pair, s]
        kT = pqk.tile([128, NP, S], F16)
        v1 = pqk.tile([128, KC, HLOC, D + 1], F16)  # [k128, kc, head, V|1]
        attnT = pqk.tile([128, NP, S], F16)
        ones1 = pp.tile([1, 128], F16)
        neg4 = pp.tile([128, 1], F32)
        nc.gpsimd.memset(neg4[:], -4.0)

        # ---- resident loads: big contiguous lines (4KB xt rows, 3KB w rows)
        # k-columns of e-chunk 0 first so the first matmul starts earliest
        nc.sync.dma_start(w_sb[:, 0, FEAT:2 * FEAT], wqkv_v[:, 0, FEAT:2 * FEAT])
        nc.sync.dma_start(xt_sb[:, 0], xt_v[:, 0])
        nc.sync.dma_start(w_sb[:, 0, 0:FEAT], wqkv_v[:, 0, 0:FEAT])
        nc.sync.dma_start(w_sb[:, 0, 2 * FEAT:], wqkv_v[:, 0, 2 * FEAT:])
        for ec in range(1, EC):
            nc.sync.dma_start(w_sb[:, ec], wqkv_v[:, ec])
            nc.sync.dma_start(xt_sb[:, ec], xt_v[:, ec])

        # constants: ones row + the ones column of [V | 1] (ACT const fill)
        nc.scalar.activation(ones1[:], w_sb[0:1, 0, 0:128],
                             AF.Copy, bias=1.0, scale=0.0)
        nc.scalar.activation(
            v1[:, :, :, D],
            w_sb[:, 0, 0:KC * HLOC].rearrange("p (a b) -> p a b", a=KC),
            AF.Copy, bias=1.0, scale=0.0)

        if has_bqkv:
            bq_sb = pp.tile([128, 4], F32)
            bk_sb = pp.tile([128, 4], F32)
            bv_row = pp.tile([1, FEAT], F16)
            nc.sync.dma_start(bq_sb[:], bq_d[:])
            nc.sync.dma_start(bk_sb[:], bk_d[:])
            nc.sync.dma_start(bv_row[:], bv_d[:])
            bv_bc = pp.tile([128, FEAT], F32)
            ps_bv = psS.tile([128, 1024], F32, tag="s", name="ps_bv")
            nc.tensor.matmul(ps_bv[:, 0:FEAT], ones1[:], bv_row[:],
                             start=True, stop=True)
            nc.vector.tensor_copy(bv_bc[:], ps_bv[:, 0:FEAT])
        if has_bp:
            bp_row = pB.tile([1, E], F16, tag="bp_row")
            bp_bc = pB.tile([128, E], F32, tag="bp_bc")
            nc.sync.dma_start(bp_row[:], bp_d[:])
            ps_bp = psS.tile([128, 1024], F32, tag="s", name="ps_bp")
            nc.tensor.matmul(ps_bp[:, 0:512], ones1[:], bp_row[:, 0:512],
                             start=True, stop=True)
            nc.tensor.matmul(ps_bp[:, 512:], ones1[:], bp_row[:, 512:],
                             start=True, stop=True)
            nc.vector.tensor_copy(bp_bc[:], ps_bp[:])

        # feature offsets into the concatenated weight
        WOFF = {"q": 0, "k": FEAT, "v": 2 * FEAT}

        def kq_unit(dst, which, fc, ch):
            """Q or K projection for one (pair, s-chunk): 8 matmuls."""
            csl = slice(ch * CH, (ch + 1) * CH)
            ps1 = psS.tile([128, 1024], F32, tag="s", name="ps1")
            off = WOFF[which] + fc * 128
            for ec in range(EC):
                nc.tensor.matmul(
                    ps1[:, 0:CH], w_sb[:, ec, off:off + 128],
                    xt_sb[:, ec, csl],
                    start=(ec == 0), stop=(ec == EC - 1))
            with nc.allow_low_precision(reason="fp16 attn"):
                if has_bqkv:
                    bias_ap = (bq_sb if which == "q" else bk_sb)[:, fc]
                    nc.scalar.activation(dst[:, fc, csl], ps1[:, 0:CH],
                                         AF.Identity, bias=bias_ap)
                else:
                    nc.vector.tensor_copy(dst[:, fc, csl], ps1[:, 0:CH])

        def v_unit(kcg):
            """V projection for one 128-token chunk: 8 matmuls."""
            ksl = slice(kcg * 128, (kcg + 1) * 128)
            ps1 = psS.tile([128, 1024], F32, tag="s", name="ps1")
            for ec in range(EC):
                nc.tensor.matmul(
                    ps1[:, 0:FEAT], xt_sb[:, ec, ksl],
                    w_sb[:, ec, 2 * FEAT:3 * FEAT],
                    start=(ec == 0), stop=(ec == EC - 1))
            ps1v = ps1[:, 0:FEAT].rearrange("p (h d) -> p h d", h=HLOC)
            with nc.allow_low_precision(reason="fp16 attn"):
                if has_bqkv:
                    bvv = bv_bc.rearrange("p (h d) -> p h d", h=HLOC)
                    nc.vector.tensor_add(v1[:, kcg, :, 0:D], ps1v, bvv)
                else:
                    nc.vector.tensor_copy(v1[:, kcg, :, 0:D], ps1v)

        def proj_unit(sc):
            """out[sc*128:+128, :] = attnT.T @ wp: 8 matmuls."""
            ssl = slice(sc * 128, (sc + 1) * 128)
            ps_p = psS.tile([128, 1024], F32, tag="s", name="ps_p")
            # fc-major: the last-normalized pair (fc3) is only needed by the
            # final two matmuls, so a unit can start while its norm finishes
            for fc in range(NP):
                for n2 in range(2):
                    nsl = slice(n2 * 512, (n2 + 1) * 512)
                    nc.tensor.matmul(ps_p[:, nsl], attnT[:, fc, ssl],
                                     wp_sb[:, fc, nsl],
                                     start=(fc == 0), stop=(fc == NP - 1))
            out_t = pout.tile([128, E], F32, tag="out", name="out_t")
            with nc.allow_low_precision(reason="fp16 attn"):
                if has_bp:
                    nc.vector.tensor_add(out_t[:], ps_p[:], bp_bc[:])
                else:
                    nc.vector.tensor_copy(out_t[:], ps_p[:])
            nc.sync.dma_start(out.ap()[ssl, :], out_t[:])

        # ---------------- attention stream ----------------
        # items: (p, qc, kc, ps_o); scores+exp issued at step i, AV at i+2
        # (lag-2 fully hides the scalar-engine exp latency + sem round trips)
        avq = deque()   # (p, qc, kc, pT, ps_o)

        def scores_part(p, qc, kc, ps_o):
            qsl = slice(qc * 512, (qc + 1) * 512)
            ks = slice(kc * 128, (kc + 1) * 128)
            ps_s = psS.tile([128, 1024], F32, tag="s", name="ps_s")
            nc.tensor.matmul(ps_s[:, 0:512],
                             kT[0:64, p, ks], qT[0:64, p, qsl],
                             start=True, stop=True)
            nc.tensor.matmul(ps_s[:, 512:],
                             kT[64:128, p, ks], qT[64:128, p, qsl],
                             start=True, stop=True)
            pT = pB.tile([128, 1024], F16, tag="pT", bufs=10, name="pT")
            # bias -4: softmax is shift-invariant; keeps exp well inside
            # fp16 range (raw scores reach ~8.3, exp(8.3)=4e3 vs max 6.5e4)
            with nc.allow_low_precision(reason="fp16 attn"):
                nc.scalar.activation(pT[:], ps_s[:], AF.Exp, scale=0.125,
                                     bias=neg4[:])
            return pT

        def av_part(p, qc, kc, pT, ps_o):
            ps_oA, ps_oB = ps_o
            nc.tensor.matmul(ps_oA[:], v1[:, kc, 2 * p, :],
                             pT[:, 0:512],
                             start=(kc == 0), stop=(kc == KC - 1))
            nc.tensor.matmul(ps_oB[:], v1[:, kc, 2 * p + 1, :],
                             pT[:, 512:],
                             start=(kc == 0), stop=(kc == KC - 1))

        def push_av(item, lag=2):
            avq.append(item)
            while len(avq) > lag:
                av_part(*avq.popleft())

        def flush_one():
            if avq:
                av_part(*avq.popleft())

        def flush_av():
            while avq:
                av_part(*avq.popleft())

        def emit(p, qc, kc, ps_o):
            pT = scores_part(p, qc, kc, ps_o)
            return (p, qc, kc, pT, ps_o)

        dens = {}

        def norm_A(p, qc, ps_o):
            """Denominator rows PSUM -> SBUF (fp16: keeps PE in fp16 mode)."""
            ps_oA, ps_oB = ps_o
            den = pB.tile([1, 1024], F16, tag="den", bufs=2, name="den")
            with nc.allow_low_precision(reason="fp16 attn"):
                nc.vector.tensor_copy(den[:, 0:512], ps_oA[D:D + 1, :])
                nc.vector.tensor_copy(den[:, 512:], ps_oB[D:D + 1, :])
            dens[(p, qc)] = den

        def norm_B(p, qc, ps_o):
            """PE-broadcast den, reciprocal on DVE, normalize into attnT."""
            ps_oA, ps_oB = ps_o
            den = dens.pop((p, qc))
            qsl = slice(qc * 512, (qc + 1) * 512)
            ps_b = psS.tile([128, 1024], F32, tag="s", name="ps_b")
            nc.tensor.matmul(ps_b[0:64, 0:512], ones1[:, 0:64],
                             den[:, 0:512], start=True, stop=True)
            nc.tensor.matmul(ps_b[0:64, 512:], ones1[:, 0:64],
                             den[:, 512:], start=True, stop=True)
            r_sb = pB.tile([64, 1024], F32, tag="r_sb", bufs=2, name="r_sb")
            nc.vector.reciprocal_approx_fast(out=r_sb[:], in_=ps_b[0:64, :])
            with nc.allow_low_precision(reason="fp16 attn"):
                nc.vector.tensor_mul(attnT[0:64, p, qsl],
                                     ps_oA[0:D, :], r_sb[:, 0:512])
                nc.vector.tensor_mul(attnT[64:128, p, qsl],
                                     ps_oB[0:D, :], r_sb[:, 512:])

        # ---------------- pass 1: fc0 k/q, all v, block (0,0) ----------
        def make_po(nm):
            return (psO.tile([D + 1, 512], F32, tag="oA", name=nm + "A"),
                    psO.tile([D + 1, 512], F32, tag="oB", name=nm + "B"))

        ps_o00 = make_po("ps_o00")

        kq_unit(kT, "k", 0, 0)
        kq_unit(qT, "q", 0, 0)
        v_unit(0)
        v_unit(1)
        # stream with interleaved units; each entry is (kc | unit-thunk)
        p1 = [0, lambda: v_unit(2), 1, lambda: v_unit(3),
              2, lambda: kq_unit(kT, "k", 0, 1),
              3, lambda: kq_unit(qT, "q", 0, 1),
              4, lambda: v_unit(4), lambda: v_unit(5),
              5, lambda: v_unit(6), lambda: v_unit(7),
              6, lambda: kq_unit(kT, "k", 0, 2),
              7, lambda: v_unit(8),
              8, lambda: v_unit(9),
              9, lambda: v_unit(10), lambda: v_unit(11),
              10, lambda: kq_unit(kT, "k", 0, 3),
              11, lambda: v_unit(12),
              12, lambda: v_unit(13),
              13, lambda: v_unit(14), lambda: v_unit(15),
              14, 15]
        for entry in p1:
            if callable(entry):
                entry()
            else:
                push_av(emit(0, 0, entry, ps_o00))
        # AV (0,0,14..15) + norm of (0,0) lag into the first main block

        nc.sync.dma_start(wp_sb[:], wp_v)

        # ---------------- main phase ----------------
        blocks = [(0, 1), (0, 2), (0, 3)] + [(p, qc) for p in range(1, NP)
                                             for qc in range(SC)]
        # cover units per block: {block: [(kc_pos, thunk), ...]}
        # deadlines: kT fc,ch before (fc, 0, kc=4*ch); qT fc,ch before (fc, ch)
        KQ = lambda d, w, fc, ch: (lambda: kq_unit(d, w, fc, ch))
        PU = lambda sc: (lambda: proj_unit(sc))
        cover = {
            (0, 1): [(3, KQ(kT, "k", 1, 0)), (7, KQ(kT, "k", 1, 1)),
                     (11, KQ(qT, "q", 0, 2))],
            (0, 2): [(3, KQ(kT, "k", 1, 2)), (7, KQ(kT, "k", 1, 3)),
                     (11, KQ(qT, "q", 0, 3))],
            (0, 3): [(3, KQ(qT, "q", 1, 0)), (11, KQ(qT, "q", 1, 1))],
            (1, 0): [(3, KQ(qT, "q", 1, 2)), (11, KQ(qT, "q", 1, 3))],
            (1, 1): [(3, KQ(kT, "k", 2, 0)), (11, KQ(kT, "k", 2, 1))],
            (1, 2): [(3, KQ(kT, "k", 2, 2)), (11, KQ(kT, "k", 2, 3))],
            (1, 3): [(3, KQ(qT, "q", 2, 0)), (11, KQ(qT, "q", 2, 1))],
            (2, 0): [(3, KQ(qT, "q", 2, 2)), (11, KQ(qT, "q", 2, 3))],
            (2, 1): [(3, KQ(kT, "k", 3, 0)), (11, KQ(kT, "k", 3, 1))],
            (2, 2): [(3, KQ(kT, "k", 3, 2)), (11, KQ(qT, "q", 3, 0))],
            (2, 3): [(3, KQ(kT, "k", 3, 3)), (11, KQ(qT, "q", 3, 1))],
            (3, 0): [(3, KQ(qT, "q", 3, 2)), (11, KQ(qT, "q", 3, 3))],
            (3, 1): [(5, PU(0)), (9, PU(1)), (12, PU(2)), (15, PU(3))],
            (3, 2): [(5, PU(4)), (9, PU(5)), (12, PU(6)), (15, PU(7))],
            (3, 3): [(5, PU(8)), (9, PU(9)), (12, PU(10)), (15, PU(11))],
        }
        pending_norm = deque()
        norm_ps = {(0, 0): ps_o00}
        pending_A = [(0, 0)]   # block whose den copy is due at next kc1

        for bi, (p, qc) in enumerate(blocks):
            ps_o = make_po("ps_o")
            norm_ps[(p, qc)] = ps_o
            cv = dict(cover[(p, qc)])
            for kc in range(KC):
                avq.append(emit(p, qc, kc, ps_o))
                # this block's AVs are held back until kc5: by then the
                # previous block's norm has freed the single-buffered O
                # accumulator, so deferred AVs never fill the PE's 4-deep
                # wait queue and never block scores/cover dispatch.
                if kc == 0:
                    flush_one()                  # AV(prev block, 14)
                if kc == 1:
                    flush_one()                  # AV(prev block, 15)
                    if pending_A[0] is not None:
                        bq_ = pending_A[0]
                        norm_A(*bq_, norm_ps[bq_])
                        pending_norm.append(bq_)
                        pending_A[0] = None
                if kc == 2 and pending_norm:
                    bq_ = pending_norm.popleft()
                    norm_B(*bq_, norm_ps.pop(bq_))
                if kc in cv:
                    cv[kc]()
                if kc >= 5:
                    flush_one()
                    if kc in (7, 10, 13):
                        flush_one()              # catch back down to lag 2
            pending_A[0] = (p, qc)

        # tail: last AVs + final norm + last projection column
        flush_av()
        norm_A(3, 3, norm_ps[(3, 3)])
        norm_B(3, 3, norm_ps.pop((3, 3)))
        for sc in range(12, 16):
            proj_unit(sc)

    nc.compile()
    return nc


def _prep_inputs(x, W_qkv, b_qkv, W_proj, b_proj, has_bqkv, has_bp):
    """Build the 8 per-core input maps (host-side sharding/layout only)."""
    Wr = np.ascontiguousarray(W_qkv.reshape(E, 3, H, D))
    in_maps = []
    for c in range(NCORES):
        b, hg = c // 2, c % 2
        hsl = slice(hg * HLOC, (hg + 1) * HLOC)
        # concatenated [q | k | v] per-core weight: [E, 3*FEAT], 3KB rows
        wqkv = np.ascontiguousarray(
            Wr[:, :, hsl, :].transpose(0, 1, 2, 3).reshape(E, 3 * FEAT)
        ).astype(np.float16)
        m = {
            "xt": np.ascontiguousarray(x[b].T).astype(np.float16),
            "wqkv": wqkv,
            "wp": np.ascontiguousarray(
                W_proj[hg * FEAT:(hg + 1) * FEAT, :]).astype(np.float16),
        }
        if has_bqkv:
            br = b_qkv.reshape(3, H, D)
            m["bq"] = np.ascontiguousarray(
                br[0, hsl, :].reshape(4, 128).T)
            m["bk"] = np.ascontiguousarray(
                br[1, hsl, :].reshape(4, 128).T)
            m["bv"] = np.ascontiguousarray(br[2, hsl, :].reshape(1, FEAT)).astype(np.float16)
        if has_bp:
            m["bp"] = np.ascontiguousarray((b_proj * 0.5).reshape(1, E)).astype(np.float16)
        in_maps.append(m)
    return in_maps


def run(x, W_qkv, b_qkv, W_proj, b_proj, trace=False):
    from concourse.bass_utils import run_bass_kernel_spmd

    has_bqkv = bool(np.any(b_qkv))
    has_bp = bool(np.any(b_proj))
    key = (has_bqkv, has_bp)
    if key not in _CACHE:
        _CACHE[key] = _build_program(has_bqkv, has_bp)
    nc = _CACHE[key]

    in_maps = _prep_inputs(x, W_qkv, b_qkv, W_proj, b_proj, has_bqkv, has_bp)
    res = run_bass_kernel_spmd(nc, in_maps, core_ids=list(range(NCORES)),
                               trace=trace)
    out = np.empty((B, S, E), dtype=np.float32)
    for b in range(B):
        out[b] = res.results[2 * b]["out"] + res.results[2 * b + 1]["out"]
    return out, res


def kernel(x, W_qkv, b_qkv, W_proj, b_proj):
    out, _ = run(np.asarray(x), np.asarray(W_qkv), np.asarray(b_qkv),
                 np.asarray(W_proj), np.asarray(b_proj))
    return out


# revision 23
# speedup vs baseline: 1.0165x; 1.0165x over previous
"""Multi-head self-attention (B=4, S=2048, E=1024, H=16, D=64) on 8 TRN2 cores.

Sharding: core c handles batch b = c//2 and head-group hg = c%2 (8 of 16 heads).
QKV weights column-parallel, proj row-parallel (Megatron); the two cores
sharing a batch produce partial proj outputs that are summed on the host.

Device layout (per core):
  xt (x[b].T, [E=1024, S=2048] fp16) and the concatenated qkv weight
  [E, 1536] are loaded ONCE into SBUF with 4KB/3KB DMA lines and stay
  resident (the v1 kernel re-fetched xt per projection unit: 42MB -> 18MB).
  qT/kT = (x W)^T per head-pair  [128 feats, pair, 2048]
  scores computed transposed:     S_T[k, q] = kT.T @ qT (pair-packed on
                                  the 64-wide contraction via base_partition)
  softmax without max-subtraction (scores ~N(0,1); exp(s/8-4) stays in
  fp16 range), denominator fused into the AV matmul via a ones-augmented
  [V | 1] stationary; O^T accumulated per head in [65, 512] PSUM tiles.
  norm: fp16 den row (keeps the PE in fp16 mode - fp32r mode switches
  cost ~0.6us PE drain each), PE ones-broadcast, DVE reciprocal + muls.
  proj: out[s, n] = attnT.T @ Wp, accumulated over feature pairs.

Scheduling: one global stream of attention (block, kc) chunks with lag-2
AV (scores run two chunks ahead of AV so the scalar-engine exp latency
and semaphore round-trips are fully hidden), 3-deep PSUM rotation for
the scores->exp pipe, and atomic 8-matmul cover units (QKV / output
projections) placed so every block has PE work while exp streams;
deferred q-column units keep the final head-pair covered too.
"""
import numpy as np

B, S, E = 4, 2048, 1024
H, D = 16, 64
HLOC = 8          # heads per core
FEAT = HLOC * D   # 512 per-core q/k/v features
NCORES = 8

_CACHE = {}


def _build_program(has_bqkv, has_bp):
    import concourse.bass as bass
    import concourse.mybir as mybir
    from concourse import bacc
    from concourse.tile import TileContext

    F32R = mybir.dt.float32r
    F32 = mybir.dt.float32
    F16 = mybir.dt.float16
    AF = mybir.ActivationFunctionType

    nc = bacc.Bacc("TRN2", target_bir_lowering=False, num_devices=NCORES)

    xt = nc.dram_tensor("xt", [E, S], F16, kind="ExternalInput")
    wqkv = nc.dram_tensor("wqkv", [128, 8 * 3 * FEAT], F16,
                          kind="ExternalInput")
    wp = nc.dram_tensor("wp", [FEAT, E], F16, kind="ExternalInput")
    out = nc.dram_tensor("out", [S, E], F32, kind="ExternalOutput")
    if has_bqkv:
        bq_d = nc.dram_tensor("bq", [128, 4], F32, kind="ExternalInput")
        bk_d = nc.dram_tensor("bk", [128, 4], F32, kind="ExternalInput")
        bv_d = nc.dram_tensor("bv", [1, FEAT], F16, kind="ExternalInput")
    if has_bp:
        bp_d = nc.dram_tensor("bp", [1, E], F16, kind="ExternalInput")

    EC = E // 128      # 8 e-chunks (contraction for qkv)
    SC = S // 512      # 4 s-chunks of 512 (q blocks)
    KC = S // 128      # 16 k-chunks of 128
    NP = HLOC // 2     # 4 head pairs
    CH = 512
    NCH = S // CH

    xt_v = xt.ap().rearrange("(c p) s -> p c s", p=128)
    wqkv_v = wqkv.ap().rearrange("p (c f) -> p c f", c=8)
    wp_v = wp.ap().rearrange("(c p) n -> p c n", p=128)

    with TileContext(nc) as tc:
      from contextlib import ExitStack
      from collections import deque
      with ExitStack() as es:
        pp = es.enter_context(tc.tile_pool(name="persist", bufs=1))
        pqk = es.enter_context(tc.tile_pool(name="pqk", bufs=1))
        pB = es.enter_context(tc.tile_pool(name="pB", bufs=1))
        pout = es.enter_context(tc.tile_pool(name="pout", bufs=3))
        psS = es.enter_context(tc.tile_pool(name="psS", bufs=3, space="PSUM"))
        psO = es.enter_context(tc.tile_pool(name="psO", bufs=1, space="PSUM"))

        xt_sb = pp.tile([128, EC, S], F16)
        w_sb = pp.tile([128, EC, 3 * FEAT], F16)   # [q | k | v] features
        wp_sb = pp.tile([128, NP, E], F16)
        qT = pqk.tile([128, NP, S], F16)     # [feat128, pair, s]
        kT = pqk.tile([128, NP, S], F16)
        v1 = pqk.tile([128, KC, HLOC, D + 1], F16)  # [k128, kc, head, V|1]
        attnT = pqk.tile([128, NP, S], F16)
        ones1 = pp.tile([1, 128], F16)
        neg4 = pp.tile([128, 1], F32)
        nc.gpsimd.memset(neg4[:], -4.0)

        # ---- resident loads. The qkv weight is host-permuted to ec-major
        # per-partition layout: one 24KB descriptor per partition moves ALL
        # weights in ~3us, so pass-1 units pace only on the 4KB xt lines.
        nc.sync.dma_start(w_sb[:], wqkv_v[:])
        for ec in range(EC):
            nc.sync.dma_start(xt_sb[:, ec], xt_v[:, ec])

        # constants: ones row + the ones column of [V | 1] (ACT const fill)
        nc.scalar.activation(ones1[:], w_sb[0:1, 0, 0:128],
                             AF.Copy, bias=1.0, scale=0.0)
        nc.scalar.activation(
            v1[:, :, :, D],
            w_sb[:, 0, 0:KC * HLOC].rearrange("p (a b) -> p a b", a=KC),
            AF.Copy, bias=1.0, scale=0.0)

        if has_bqkv:
            bq_sb = pp.tile([128, 4], F32)
            bk_sb = pp.tile([128, 4], F32)
            bv_row = pp.tile([1, FEAT], F16)
            nc.sync.dma_start(bq_sb[:], bq_d[:])
            nc.sync.dma_start(bk_sb[:], bk_d[:])
            nc.sync.dma_start(bv_row[:], bv_d[:])
            bv_bc = pp.tile([128, FEAT], F32)
            ps_bv = psS.tile([128, 1024], F32, tag="s", name="ps_bv")
            nc.tensor.matmul(ps_bv[:, 0:FEAT], ones1[:], bv_row[:],
                             start=True, stop=True)
            nc.vector.tensor_copy(bv_bc[:], ps_bv[:, 0:FEAT])
        if has_bp:
            bp_row = pB.tile([1, E], F16, tag="bp_row")
            bp_bc = pB.tile([128, E], F32, tag="bp_bc")
            nc.sync.dma_start(bp_row[:], bp_d[:])
            ps_bp = psS.tile([128, 1024], F32, tag="s", name="ps_bp")
            nc.tensor.matmul(ps_bp[:, 0:512], ones1[:], bp_row[:, 0:512],
                             start=True, stop=True)
            nc.tensor.matmul(ps_bp[:, 512:], ones1[:], bp_row[:, 512:],
                             start=True, stop=True)
            nc.vector.tensor_copy(bp_bc[:], ps_bp[:])

        # feature offsets into the concatenated weight
        WOFF = {"q": 0, "k": FEAT, "v": 2 * FEAT}

        def kq_unit(dst, which, fc, ch):
            """Q or K projection for one (pair, s-chunk): 8 matmuls."""
            csl = slice(ch * CH, (ch + 1) * CH)
            ps1 = psS.tile([128, 1024], F32, tag="s", name="ps1")
            off = WOFF[which] + fc * 128
            for ec in range(EC):
                nc.tensor.matmul(
                    ps1[:, 0:CH], w_sb[:, ec, off:off + 128],
                    xt_sb[:, ec, csl],
                    start=(ec == 0), stop=(ec == EC - 1))
            with nc.allow_low_precision(reason="fp16 attn"):
                if has_bqkv:
                    bias_ap = (bq_sb if which == "q" else bk_sb)[:, fc]
                    nc.scalar.activation(dst[:, fc, csl], ps1[:, 0:CH],
                                         AF.Identity, bias=bias_ap)
                else:
                    nc.vector.tensor_copy(dst[:, fc, csl], ps1[:, 0:CH])

        def v_unit(kcg):
            """V projection for one 128-token chunk: 8 matmuls."""
            ksl = slice(kcg * 128, (kcg + 1) * 128)
            ps1 = psS.tile([128, 1024], F32, tag="s", name="ps1")
            for ec in range(EC):
                nc.tensor.matmul(
                    ps1[:, 0:FEAT], xt_sb[:, ec, ksl],
                    w_sb[:, ec, 2 * FEAT:3 * FEAT],
                    start=(ec == 0), stop=(ec == EC - 1))
            ps1v = ps1[:, 0:FEAT].rearrange("p (h d) -> p h d", h=HLOC)
            with nc.allow_low_precision(reason="fp16 attn"):
                if has_bqkv:
                    bvv = bv_bc.rearrange("p (h d) -> p h d", h=HLOC)
                    nc.vector.tensor_add(v1[:, kcg, :, 0:D], ps1v, bvv)
                else:
                    nc.vector.tensor_copy(v1[:, kcg, :, 0:D], ps1v)

        def proj_unit(sc):
            """out[sc*128:+128, :] = attnT.T @ wp: 8 matmuls."""
            ssl = slice(sc * 128, (sc + 1) * 128)
            ps_p = psS.tile([128, 1024], F32, tag="s", name="ps_p")
            for n2 in range(2):
                nsl = slice(n2 * 512, (n2 + 1) * 512)
                for fc in range(NP):
                    nc.tensor.matmul(ps_p[:, nsl], attnT[:, fc, ssl],
                                     wp_sb[:, fc, nsl],
                                     start=(fc == 0), stop=(fc == NP - 1))
            out_t = pout.tile([128, E], F32, tag="out", name="out_t")
            with nc.allow_low_precision(reason="fp16 attn"):
                if has_bp:
                    nc.vector.tensor_add(out_t[:], ps_p[:], bp_bc[:])
                else:
                    nc.vector.tensor_copy(out_t[:], ps_p[:])
            nc.sync.dma_start(out.ap()[ssl, :], out_t[:])

        # ---------------- attention stream ----------------
        # items: (p, qc, kc, ps_o); scores+exp issued at step i, AV at i+2
        # (lag-2 fully hides the scalar-engine exp latency + sem round trips)
        avq = deque()   # (p, qc, kc, pT, ps_o)

        def scores_part(p, qc, kc, ps_o):
            qsl = slice(qc * 512, (qc + 1) * 512)
            ks = slice(kc * 128, (kc + 1) * 128)
            ps_s = psS.tile([128, 1024], F32, tag="s", name="ps_s")
            nc.tensor.matmul(ps_s[:, 0:512],
                             kT[0:64, p, ks], qT[0:64, p, qsl],
                             start=True, stop=True)
            nc.tensor.matmul(ps_s[:, 512:],
                             kT[64:128, p, ks], qT[64:128, p, qsl],
                             start=True, stop=True)
            pT = pB.tile([128, 1024], F16, tag="pT", bufs=10, name="pT")
            # bias -4: softmax is shift-invariant; keeps exp well inside
            # fp16 range (raw scores reach ~8.3, exp(8.3)=4e3 vs max 6.5e4)
            with nc.allow_low_precision(reason="fp16 attn"):
                nc.scalar.activation(pT[:], ps_s[:], AF.Exp, scale=0.125,
                                     bias=neg4[:])
            return pT

        def av_part(p, qc, kc, pT, ps_o):
            ps_oA, ps_oB = ps_o
            nc.tensor.matmul(ps_oA[:], v1[:, kc, 2 * p, :],
                             pT[:, 0:512],
                             start=(kc == 0), stop=(kc == KC - 1))
            nc.tensor.matmul(ps_oB[:], v1[:, kc, 2 * p + 1, :],
                             pT[:, 512:],
                             start=(kc == 0), stop=(kc == KC - 1))

        def push_av(item, lag=2):
            avq.append(item)
            while len(avq) > lag:
                av_part(*avq.popleft())

        def flush_one():
            if avq:
                av_part(*avq.popleft())

        def flush_av():
            while avq:
                av_part(*avq.popleft())

        def emit(p, qc, kc, ps_o):
            pT = scores_part(p, qc, kc, ps_o)
            return (p, qc, kc, pT, ps_o)

        dens = {}

        def norm_A(p, qc, ps_o):
            """Denominator rows PSUM -> SBUF (fp16: keeps PE in fp16 mode)."""
            ps_oA, ps_oB = ps_o
            den = pB.tile([1, 1024], F16, tag="den", bufs=2, name="den")
            with nc.allow_low_precision(reason="fp16 attn"):
                nc.vector.tensor_copy(den[:, 0:512], ps_oA[D:D + 1, :])
                nc.vector.tensor_copy(den[:, 512:], ps_oB[D:D + 1, :])
            dens[(p, qc)] = den

        def norm_B(p, qc, ps_o):
            """PE-broadcast den, reciprocal on DVE, normalize into attnT."""
            ps_oA, ps_oB = ps_o
            den = dens.pop((p, qc))
            qsl = slice(qc * 512, (qc + 1) * 512)
            ps_b = psS.tile([128, 1024], F32, tag="s", name="ps_b")
            nc.tensor.matmul(ps_b[0:64, 0:512], ones1[:, 0:64],
                             den[:, 0:512], start=True, stop=True)
            nc.tensor.matmul(ps_b[0:64, 512:], ones1[:, 0:64],
                             den[:, 512:], start=True, stop=True)
            r_sb = pB.tile([64, 1024], F32, tag="r_sb", bufs=2, name="r_sb")
            nc.vector.reciprocal_approx_fast(out=r_sb[:], in_=ps_b[0:64, :])
            with nc.allow_low_precision(reason="fp16 attn"):
                nc.vector.tensor_mul(attnT[0:64, p, qsl],
                                     ps_oA[0:D, :], r_sb[:, 0:512])
                nc.vector.tensor_mul(attnT[64:128, p, qsl],
                                     ps_oB[0:D, :], r_sb[:, 512:])

        # ---------------- pass 1: fc0 k/q, all v, block (0,0) ----------
        def make_po(nm):
            return (psO.tile([D + 1, 512], F32, tag="oA", name=nm + "A"),
                    psO.tile([D + 1, 512], F32, tag="oB", name=nm + "B"))

        ps_o00 = make_po("ps_o00")

        kq_unit(kT, "k", 0, 0)
        kq_unit(qT, "q", 0, 0)
        v_unit(0)
        v_unit(1)
        # stream with interleaved units; each entry is (kc | unit-thunk)
        p1 = [0, lambda: v_unit(2), 1, lambda: v_unit(3),
              2, lambda: kq_unit(kT, "k", 0, 1),
              3, lambda: kq_unit(qT, "q", 0, 1),
              4, lambda: v_unit(4), lambda: v_unit(5),
              5, lambda: v_unit(6), lambda: v_unit(7),
              6, lambda: kq_unit(kT, "k", 0, 2),
              7, lambda: v_unit(8),
              8, lambda: v_unit(9),
              9, lambda: v_unit(10), lambda: v_unit(11),
              10, lambda: kq_unit(kT, "k", 0, 3),
              11, lambda: v_unit(12),
              12, lambda: v_unit(13),
              13, lambda: v_unit(14), lambda: v_unit(15),
              14, 15]
        for entry in p1:
            if callable(entry):
                entry()
            else:
                push_av(emit(0, 0, entry, ps_o00))
        # AV (0,0,14..15) + norm of (0,0) lag into the first main block

        nc.sync.dma_start(wp_sb[:], wp_v)

        # ---------------- main phase ----------------
        blocks = [(0, 1), (0, 2), (0, 3)] + [(p, qc) for p in range(1, NP)
                                             for qc in range(SC)]
        # cover units per block: {block: [(kc_pos, thunk), ...]}
        # deadlines: kT fc,ch before (fc, 0, kc=4*ch); qT fc,ch before (fc, ch)
        KQ = lambda d, w, fc, ch: (lambda: kq_unit(d, w, fc, ch))
        PU = lambda sc: (lambda: proj_unit(sc))
        cover = {
            (0, 1): [(3, KQ(kT, "k", 1, 0)), (7, KQ(kT, "k", 1, 1)),
                     (11, KQ(qT, "q", 0, 2))],
            (0, 2): [(3, KQ(kT, "k", 1, 2)), (7, KQ(kT, "k", 1, 3)),
                     (11, KQ(qT, "q", 0, 3))],
            (0, 3): [(3, KQ(qT, "q", 1, 0)), (11, KQ(qT, "q", 1, 1))],
            (1, 0): [(3, KQ(qT, "q", 1, 2)), (11, KQ(qT, "q", 1, 3))],
            (1, 1): [(3, KQ(kT, "k", 2, 0)), (11, KQ(kT, "k", 2, 1))],
            (1, 2): [(3, KQ(kT, "k", 2, 2)), (11, KQ(kT, "k", 2, 3))],
            (1, 3): [(3, KQ(qT, "q", 2, 0)), (11, KQ(qT, "q", 2, 1))],
            (2, 0): [(3, KQ(qT, "q", 2, 2)), (11, KQ(qT, "q", 2, 3))],
            (2, 1): [(3, KQ(kT, "k", 3, 0)), (11, KQ(kT, "k", 3, 1))],
            (2, 2): [(3, KQ(kT, "k", 3, 2)), (11, KQ(qT, "q", 3, 0))],
            (2, 3): [(3, KQ(kT, "k", 3, 3)), (11, KQ(qT, "q", 3, 1))],
            (3, 0): [(3, KQ(qT, "q", 3, 2)), (11, KQ(qT, "q", 3, 3))],
            (3, 1): [(5, PU(0)), (9, PU(1)), (12, PU(2)), (15, PU(3))],
            (3, 2): [(5, PU(4)), (9, PU(5)), (12, PU(6)), (15, PU(7))],
            (3, 3): [(5, PU(8)), (9, PU(9)), (12, PU(10)), (15, PU(11))],
        }
        pending_norm = deque()
        norm_ps = {(0, 0): ps_o00}
        pending_A = [(0, 0)]   # block whose den copy is due at next kc1

        for bi, (p, qc) in enumerate(blocks):
            ps_o = make_po("ps_o")
            norm_ps[(p, qc)] = ps_o
            cv = dict(cover[(p, qc)])
            for kc in range(KC):
                avq.append(emit(p, qc, kc, ps_o))
                # this block's AVs are held back until kc5: by then the
                # previous block's norm has freed the single-buffered O
                # accumulator, so deferred AVs never fill the PE's 4-deep
                # wait queue and never block scores/cover dispatch.
                if kc == 0:
                    flush_one()                  # AV(prev block, 14)
                if kc == 1:
                    flush_one()                  # AV(prev block, 15)
                    if pending_A[0] is not None:
                        bq_ = pending_A[0]
                        norm_A(*bq_, norm_ps[bq_])
                        pending_norm.append(bq_)
                        pending_A[0] = None
                if kc == 2 and pending_norm:
                    bq_ = pending_norm.popleft()
                    norm_B(*bq_, norm_ps.pop(bq_))
                if kc in cv:
                    cv[kc]()
                if kc >= 5:
                    flush_one()
                    if kc in (7, 10, 13):
                        flush_one()              # catch back down to lag 2
            pending_A[0] = (p, qc)

        # tail: last AVs + final norm + last projection column
        flush_av()
        norm_A(3, 3, norm_ps[(3, 3)])
        norm_B(3, 3, norm_ps.pop((3, 3)))
        for sc in range(12, 16):
            proj_unit(sc)

    nc.compile()
    return nc


def _prep_inputs(x, W_qkv, b_qkv, W_proj, b_proj, has_bqkv, has_bp):
    """Build the 8 per-core input maps (host-side sharding/layout only)."""
    Wr = np.ascontiguousarray(W_qkv.reshape(E, 3, H, D))
    in_maps = []
    for c in range(NCORES):
        b, hg = c // 2, c % 2
        hsl = slice(hg * HLOC, (hg + 1) * HLOC)
        # concatenated [q | k | v] per-core weight: [E, 3*FEAT], 3KB rows
        wq_flat = Wr[:, :, hsl, :].reshape(E, 3 * FEAT)
        wqkv = np.ascontiguousarray(
            wq_flat.reshape(8, 128, 3 * FEAT).transpose(1, 0, 2)
            .reshape(128, 8 * 3 * FEAT)).astype(np.float16)
        m = {
            "xt": np.ascontiguousarray(x[b].T).astype(np.float16),
            "wqkv": wqkv,
            "wp": np.ascontiguousarray(
                W_proj[hg * FEAT:(hg + 1) * FEAT, :]).astype(np.float16),
        }
        if has_bqkv:
            br = b_qkv.reshape(3, H, D)
            m["bq"] = np.ascontiguousarray(
                br[0, hsl, :].reshape(4, 128).T)
            m["bk"] = np.ascontiguousarray(
                br[1, hsl, :].reshape(4, 128).T)
            m["bv"] = np.ascontiguousarray(br[2, hsl, :].reshape(1, FEAT)).astype(np.float16)
        if has_bp:
            m["bp"] = np.ascontiguousarray((b_proj * 0.5).reshape(1, E)).astype(np.float16)
        in_maps.append(m)
    return in_maps


def run(x, W_qkv, b_qkv, W_proj, b_proj, trace=False):
    from concourse.bass_utils import run_bass_kernel_spmd

    has_bqkv = bool(np.any(b_qkv))
    has_bp = bool(np.any(b_proj))
    key = (has_bqkv, has_bp)
    if key not in _CACHE:
        _CACHE[key] = _build_program(has_bqkv, has_bp)
    nc = _CACHE[key]

    in_maps = _prep_inputs(x, W_qkv, b_qkv, W_proj, b_proj, has_bqkv, has_bp)
    res = run_bass_kernel_spmd(nc, in_maps, core_ids=list(range(NCORES)),
                               trace=trace)
    out = np.empty((B, S, E), dtype=np.float32)
    for b in range(B):
        out[b] = res.results[2 * b]["out"] + res.results[2 * b + 1]["out"]
    return out, res


def kernel(x, W_qkv, b_qkv, W_proj, b_proj):
    out, _ = run(np.asarray(x), np.asarray(W_qkv), np.asarray(b_qkv),
                 np.asarray(W_proj), np.asarray(b_proj))
    return out
